# revision 5
# baseline (speedup 1.0000x reference)
"""Adaptive embedding lookup (4 vocab buckets, per-bucket projection) on 8 TRN2 cores.

Strategy: token-parallel SPMD, bulk SWDGE gathers + XBAR DMA transposes.

Host side: tokens are bucketed by vocab range, sorted by table row, and dealt
to the 8 cores as balanced *contiguous* chunks of the sorted order, so each
core's rows for a bucket span a narrow window of the table. Each core gets its
own bf16 copy of exactly that window uploaded as an input, which keeps gather
indices within int16 range regardless of vocab size. Projections are
pre-transposed, EMB_SCALE-folded, and packed into two bf16 SBUF images.

Device side (per core):
  - one SWDGE dma_gather per bucket segment (<=896 idx each; larger counts
    crash the ucode): fetches all of the segment's rows in one ~1.3us
    instruction as contiguous 256B..2KB row writes, tokens on partitions.
    Buckets with rows < 256B gather a 256B element spanning several
    consecutive rows; downstream just ignores the trailing junk.
  - one XBAR dma transpose (sync HWDGE) per segment flips every [128,128]
    block to embed-dim-major in ~14ns per 16x128 tile -- the PE does no
    transposes at all.
  - bf16 matmuls against the packed projections, accumulating in PSUM
  - PSUM -> SBUF bf16 casts split across Vector/Scalar into one persistent
    output image [128, T, 1024], written back with one big DMA per segment
A short burst of dummy matmuls at graph start ramps the PE p-state clock
(0.65 -> 1.2 -> 2.4 GHz after 3us busy) while the first gathers land.
Host inverse-permutes the 8 bf16 shards into the full f32 output.
"""
import sys

import numpy as np

if "/opt/trn_rl_repo" not in sys.path:
    sys.path.insert(0, "/opt/trn_rl_repo")

import ml_dtypes  # noqa: E402
from concourse import bacc, bass, mybir, tile  # noqa: E402
from concourse.bass_utils import run_bass_kernel_spmd  # noqa: E402

N_CORES = 8
P = 128
CUTS = [0, 20000, 40000, 200000, 267735]
N_BUCKETS = 4
D_PROJ = 1024
EMB_SCALE = float(D_PROJ) ** 0.5
D_EMB = [1024, 256, 64, 16]
ELEM = [1024, 256, 128, 128]  # gather element size (bf16 elems), >=256B each
RPE = [1, 1, 2, 8]  # consecutive table rows packed per window row
IDX_SPAN = 32000  # max rows one gather segment may span (int16 headroom)
SEG_CAP = 896  # max indices per dma_gather (HW ucode fails in (896, 1024])
FIRST_SEG_CAP = 256  # small first segment: earliest possible first matmul

F32 = mybir.dt.float32
BF16 = mybir.dt.bfloat16
I16 = mybir.dt.int16
BF16NP = ml_dtypes.bfloat16

# compute/gather order: b2 first (most tiles, smallest proj dependency),
# b0 last (needs the 2MB ptB image, which streams in behind ptA)
BUCKET_ORDER = [2, 3, 1, 0]


def _cdiv(a, b):
    return -(-a // b)


def _build_graph(plan):
    nc = bacc.Bacc(None, target_bir_lowering=False, debug=False)

    C = plan["idx_cols"]
    idx_p = nc.declare_dram_parameter("idx", [P, C], I16, isOutput=False)
    w_p = {}
    for (b, s) in plan["segs"]:
        w_p[(b, s)] = nc.declare_dram_parameter(
            f"w{b}_{s}", [plan["W"][(b, s)], ELEM[b]], BF16, isOutput=False
        )
    ptA_p = nc.declare_dram_parameter("ptA", [P, 4096], BF16, isOutput=False)
    ptB_p = nc.declare_dram_parameter("ptB", [P, 8 * 1024], BF16, isOutput=False)
    T = plan["tiles_total"]
    out_p = nc.declare_dram_parameter("out", [P, T, D_PROJ], BF16, isOutput=True)

    with tile.TileContext(nc) as tc:
        with (
            tc.tile_pool(name="persist", bufs=1) as pp,
            tc.tile_pool(name="ps_mm", bufs=3, space="PSUM") as ps_mm,
            tc.tile_pool(name="ps_warm", bufs=1, space="PSUM") as ps_warm,
        ):
            # idx load first on the sync HWDGE queue; everything hangs off it
            idx_sb = pp.tile([P, C], I16)
            nc.sync.dma_start(out=idx_sb[:], in_=idx_p[:])

            # PE warmup: ramp the p-state clock while the first gather lands
            warm = pp.tile([P, 512], BF16, tag="warm")
            nc.vector.memset(warm[:], 0)
            wps = ps_warm.tile([P, 512], F32, tag="warm_ps")
            for _ in range(12):
                nc.tensor.matmul(wps[:], warm[:, :P], warm[:], start=True, stop=True)

            ptA_sb = pp.tile([P, 4096], BF16, tag="ptA")
            nc.scalar.dma_start(out=ptA_sb[:], in_=ptA_p[:])
            ptB_sb = pp.tile([P, 8 * 1024], BF16, tag="ptB")
            nc.scalar.dma_start(out=ptB_sb[:], in_=ptB_p[:])

            # bulk gathers (gpsimd SWDGE), then one XBAR transpose per segment
            g_sb, l_sb = {}, {}
            for (b, s) in plan["segs"]:
                kx = ELEM[b] // P
                N = plan["N"][(b, s)]
                nt = N // P
                g = pp.tile([P, nt, ELEM[b]], BF16, tag=f"g{b}_{s}")
                o = plan["idx_off"][(b, s)]
                nc.gpsimd.dma_gather(
                    g[:, :, :],
                    w_p[(b, s)][:, :],
                    idx_sb[:, o : o + N // 16],
                    N,
                    N,
                    ELEM[b],
                    transpose=False,
                )
                g_sb[(b, s)] = g
            for (b, s) in plan["segs"]:
                kx = ELEM[b] // P
                N = plan["N"][(b, s)]
                nt = N // P
                lhsT = pp.tile([P, nt * kx, P], BF16, tag=f"l{b}_{s}")
                nc.sync.dma_start(
                    out=lhsT[:, :, :], in_=g_sb[(b, s)][:, :, :], transpose=True
                )
                l_sb[(b, s)] = lhsT

            # persistent output image, one big writeback per segment
            obuf = pp.tile([P, T * D_PROJ], BF16, tag="obuf")

            for (b, s) in plan["segs"]:
                N = plan["N"][(b, s)]
                nt = N // P
                kx = ELEM[b] // P
                lhsT = l_sb[(b, s)]
                d = D_EMB[b]
                kc = _cdiv(d, P)
                pt_sb = ptB_sb if b == 0 else ptA_sb
                pt_off = plan["pt_off"][b]
                t0 = plan["tile_off"][(b, s)]
                for j in range(nt):
                    mm0 = ps_mm.tile([P, 512], F32, tag="mm0")
                    mm1 = ps_mm.tile([P, 512], F32, tag="mm1")
                    mms = [mm0, mm1]
                    for k in range(kc):
                        cw = min(P, d - k * P)
                        lk = lhsT[0:cw, j * kx + k, :]
                        for h in range(2):
                            nc.tensor.matmul(
                                mms[h][:, :],
                                lk,
                                pt_sb[0:cw, pt_off + k * 1024 + h * 512 : pt_off + k * 1024 + (h + 1) * 512],
                                start=(k == 0),
                                stop=(k == kc - 1),
                            )
                    ob = (t0 + j) * D_PROJ
                    nc.vector.tensor_copy(out=obuf[:, ob : ob + 512], in_=mm0[:, :])
                    nc.scalar.activation(
                        out=obuf[:, ob + 512 : ob + 1024],
                        in_=mm1[:, :],
                        func=mybir.ActivationFunctionType.Copy,
                    )
                nc.sync.dma_start(
                    out=out_p[:, t0 : t0 + nt, :],
                    in_=obuf[:, t0 * D_PROJ : (t0 + nt) * D_PROJ],
                )

    nc.compile()
    return nc


def _make_windows(table_bf, start, W, rpe):
    """Rows [start, start+W) of the rpe-rows-per-element packed view of
    table_bf, zero-padded past the table end."""
    v, d = table_bf.shape
    out = np.zeros((W, rpe * d), dtype=BF16NP)
    take = min(W, v - start)
    if rpe == 1:
        out[:take] = table_bf[start : start + take]
        return out
    tbp = table_bf[start : start + take + rpe - 1]
    if tbp.shape[0] < take + rpe - 1:
        tbp = np.concatenate(
            [tbp, np.zeros((take + rpe - 1 - tbp.shape[0], d), dtype=BF16NP)]
        )
    sw = np.lib.stride_tricks.sliding_window_view(tbp, rpe, axis=0)  # [take, d, rpe]
    out[:take] = np.ascontiguousarray(sw.transpose(0, 2, 1)).reshape(take, rpe * d)
    return out


def kernel(inp, emb0, emb1, emb2, emb3, proj0, proj1, proj2, proj3):
    embs = [np.asarray(e, dtype=np.float32) for e in (emb0, emb1, emb2, emb3)]
    projs = [proj0, proj1, proj2, proj3]
    v_emb = [e.shape[0] for e in embs]
    embs_bf = [e.astype(BF16NP) for e in embs]

    inp = np.asarray(inp)
    orig_shape = inp.shape
    flat = inp.reshape(-1).astype(np.int64)

    bucket = np.digitize(flat, CUTS[1:-1])  # 0..3
    local = flat - np.asarray(CUTS, dtype=np.int64)[bucket]

    # per bucket: sort by row, deal balanced contiguous chunks to cores,
    # then greedy-split each core's chunk into segments bounded by IDX_SPAN
    # rows and SEG_CAP indices (first segment small for fast pipeline start)
    core_segs = {b: [[] for _ in range(N_CORES)] for b in range(N_BUCKETS)}
    first_bucket = BUCKET_ORDER[0]
    for b in range(N_BUCKETS):
        pos = np.nonzero(bucket == b)[0]
        loc = np.clip(local[pos], 0, v_emb[b] - 1)
        srt = np.argsort(loc, kind="stable")
        pos, loc = pos[srt], loc[srt]
        n = len(pos)
        base, rem = divmod(n, N_CORES)
        ofs = 0
        for c in range(N_CORES):
            cnt = base + (1 if c < rem else 0)
            lc, pc = loc[ofs : ofs + cnt], pos[ofs : ofs + cnt]
            ofs += cnt
            segs = []
            i = 0
            while i < len(lc):
                cap = FIRST_SEG_CAP if (b == first_bucket and i == 0) else SEG_CAP
                start = int(lc[i])
                jend = min(
                    int(np.searchsorted(lc, start + IDX_SPAN, side="left")),
                    i + cap,
                )
                segs.append((start, lc[i:jend], pc[i:jend]))
                i = jend
            if not segs:
                segs = [(0, lc[:0], pc[:0])]
            core_segs[b][c] = segs

    # uniform SPMD shapes: per bucket, G segments; per segment, N idx slots
    # (multiple of 128, padded with idx 0) and W window rows (max span)
    plan = {"segs": [], "N": {}, "W": {}, "idx_off": {}, "tile_off": {}}
    for b in BUCKET_ORDER:
        G = max(len(core_segs[b][c]) for c in range(N_CORES))
        for c in range(N_CORES):
            while len(core_segs[b][c]) < G:
                core_segs[b][c].append((0, np.zeros(0, np.int64), np.zeros(0, np.int64)))
        for s in range(G):
            plan["segs"].append((b, s))
            maxn = max(len(core_segs[b][c][s][1]) for c in range(N_CORES))
            plan["N"][(b, s)] = max(P, _cdiv(maxn, P) * P)
            maxw = 1
            for c in range(N_CORES):
                st, lc, _ = core_segs[b][c][s]
                if len(lc):
                    maxw = max(maxw, int(lc[-1]) - st + 1)
            plan["W"][(b, s)] = maxw

    co = 0
    to = 0
    for (b, s) in plan["segs"]:
        plan["idx_off"][(b, s)] = co
        plan["tile_off"][(b, s)] = to
        co += plan["N"][(b, s)] // 16
        to += plan["N"][(b, s)] // P
    plan["idx_cols"] = co
    plan["tiles_total"] = to

    # packed projection images: ptA = [b2 | b3 | b1 chunks], ptB = b0 chunks
    pt_scaled = [
        (np.asarray(projs[b], dtype=np.float32).T * EMB_SCALE) for b in range(N_BUCKETS)
    ]  # [d_b, 1024]
    plan["pt_off"] = {2: 0, 3: 1024, 1: 2048, 0: 0}
    ptA = np.zeros((P, 4096), dtype=np.float32)
    ptA[0:64, 0:1024] = pt_scaled[2]
    ptA[0:16, 1024:2048] = pt_scaled[3]
    ptA[:, 2048:3072] = pt_scaled[1][0:128]
    ptA[:, 3072:4096] = pt_scaled[1][128:256]
    ptB = np.zeros((P, 8 * 1024), dtype=np.float32)
    for k in range(8):
        ptB[:, k * 1024 : (k + 1) * 1024] = pt_scaled[0][k * P : (k + 1) * P]
    ptA = ptA.astype(BF16NP)
    ptB = ptB.astype(BF16NP)

    nc = _build_graph(plan)

    in_maps = []
    for c in range(N_CORES):
        im = {"ptA": ptA, "ptB": ptB}
        idx_img = np.zeros((P, plan["idx_cols"]), dtype=np.int16)
        for (b, s) in plan["segs"]:
            st, lc, _ = core_segs[b][c][s]
            N = plan["N"][(b, s)]
            rel = np.zeros(N, dtype=np.int16)
            rel[: len(lc)] = (lc - st).astype(np.int16)
            o = plan["idx_off"][(b, s)]
            wrapped = rel.reshape(N // 16, 16).T  # [16, N/16]
            idx_img[:, o : o + N // 16] = np.tile(wrapped, (8, 1))
            im[f"w{b}_{s}"] = _make_windows(embs_bf[b], st, plan["W"][(b, s)], RPE[b])
        im["idx"] = idx_img
        in_maps.append(im)

    res = run_bass_kernel_spmd(nc, in_maps, core_ids=list(range(N_CORES)))

    out_full = np.zeros((flat.shape[0], D_PROJ), dtype=np.float32)
    for c in range(N_CORES):
        shard = np.asarray(res.results[c]["out"])  # [128, T, 1024] bf16
        for (b, s) in plan["segs"]:
            _, lc, pc = core_segs[b][c][s]
            if len(pc):
                t0 = plan["tile_off"][(b, s)]
                nt = plan["N"][(b, s)] // P
                blk = (
                    shard[:, t0 : t0 + nt, :]
                    .transpose(1, 0, 2)
                    .reshape(nt * P, D_PROJ)[: len(pc)]
                )
                out_full[pc] = blk.astype(np.float32)
    return out_full.reshape(*orig_shape, D_PROJ)


# revision 6
# speedup vs baseline: 1.2682x; 1.2682x over previous
"""Adaptive embedding lookup (4 vocab buckets, per-bucket projection) on 8 TRN2 cores.

Strategy: token-parallel SPMD, bf16 end-to-end, per-tile indirect gathers.

Host side: tokens are bucketed by vocab range, sorted by table row, and dealt
to the 8 cores as balanced *contiguous* chunks of the sorted order. Each core
gets a bf16 copy of exactly its span of each table (a "window") uploaded as an
input; gather indices are window-relative int32. Projections are
pre-transposed, EMB_SCALE-folded, and packed into two bf16 images.

Device side (per core):
  - per 128-token tile, one SWDGE indirect DMA gathers the tile's bf16 rows
    (~1.1us fixed engine cost each -- the pipeline bottleneck, overlapped
    with everything else)
  - PE transposes each gathered [128, d] tile (bf16: 1 cycle/row) and
    bf16 matmuls against the packed projections; PE has slack vs the gathers
  - PSUM -> SBUF bf16 casts split across Vector/Scalar into one persistent
    output image [128, T, 1024], written back with one DMA per bucket
A burst of dummy matmuls at graph start ramps the PE p-state clock
(0.65 -> 1.2 -> 2.4 GHz after 3us busy) while the first gathers land.
Host inverse-permutes the 8 bf16 shards into the full f32 output.
"""
import sys

import numpy as np

if "/opt/trn_rl_repo" not in sys.path:
    sys.path.insert(0, "/opt/trn_rl_repo")

import ml_dtypes  # noqa: E402
from concourse import bacc, bass, mybir, tile  # noqa: E402
from concourse.bass_utils import run_bass_kernel_spmd  # noqa: E402
from concourse.masks import make_identity  # noqa: E402

N_CORES = 8
P = 128
CUTS = [0, 20000, 40000, 200000, 267735]
N_BUCKETS = 4
D_PROJ = 1024
EMB_SCALE = float(D_PROJ) ** 0.5
D_EMB = [1024, 256, 64, 16]

F32 = mybir.dt.float32
BF16 = mybir.dt.bfloat16
I32 = mybir.dt.int32
BF16NP = ml_dtypes.bfloat16

# compute/gather order: b2 first (most tiles, smallest proj dependency),
# b0 last (needs the 2MB ptB image, which streams in behind ptA)
BUCKET_ORDER = [2, 3, 1, 0]


def _cdiv(a, b):
    return -(-a // b)


def _build_graph(plan):
    nc = bacc.Bacc(None, target_bir_lowering=False, debug=False)

    T = plan["tiles_total"]
    idx_p = nc.declare_dram_parameter("idx", [P, T], I32, isOutput=False)
    w_p = {}
    for b in range(N_BUCKETS):
        w_p[b] = nc.declare_dram_parameter(
            f"w{b}", [plan["W"][b], D_EMB[b]], BF16, isOutput=False
        )
    ptA_p = nc.declare_dram_parameter("ptA", [P, 4096], BF16, isOutput=False)
    ptB_p = nc.declare_dram_parameter("ptB", [P, 8 * 1024], BF16, isOutput=False)
    out_p = nc.declare_dram_parameter("out", [P, T, D_PROJ], BF16, isOutput=True)

    with tile.TileContext(nc) as tc:
        with (
            tc.tile_pool(name="persist", bufs=1) as pp,
            tc.tile_pool(name="gather", bufs=6) as gp,
            tc.tile_pool(name="lhsT", bufs=4) as lp,
            tc.tile_pool(name="ps_tr", bufs=2, space="PSUM") as ps_tr,
            tc.tile_pool(name="ps_mm", bufs=2, space="PSUM") as ps_mm,
            tc.tile_pool(name="ps_warm", bufs=1, space="PSUM") as ps_warm,
        ):
            # idx load first, on the gpsimd queue: the gathers that need it
            # are on the same engine FIFO, avoiding cross-queue sem latency
            idx_sb = pp.tile([P, T], I32)
            nc.gpsimd.dma_start(out=idx_sb[:], in_=idx_p[:])

            ident = pp.tile([P, P], BF16)
            make_identity(nc, ident[:])

            # PE warmup: ramp the p-state clock while the first gathers land
            warm = pp.tile([P, 512], BF16, tag="warm")
            nc.vector.memset(warm[:], 0)
            wps = ps_warm.tile([P, 512], F32, tag="warm_ps")
            for _ in range(10):
                nc.tensor.matmul(wps[:], warm[:, :P], warm[:], start=True, stop=True)

            ptA_sb = pp.tile([P, 4096], BF16, tag="ptA")
            nc.scalar.dma_start(out=ptA_sb[:], in_=ptA_p[:])
            ptB_sb = pp.tile([P, 8 * 1024], BF16, tag="ptB")
            nc.scalar.dma_start(out=ptB_sb[:], in_=ptB_p[:])

            # persistent output image, one big writeback per bucket
            obuf = pp.tile([P, T * D_PROJ], BF16, tag="obuf")

            ncast = 0
            for b in BUCKET_ORDER:
                d = D_EMB[b]
                kc = _cdiv(d, P)
                nt = plan["N"][b] // P
                t0 = plan["tile_off"][b]
                pt_sb = ptB_sb if b == 0 else ptA_sb
                pt_off = plan["pt_off"][b]
                for j in range(nt):
                    t = t0 + j
                    g = gp.tile([P, d], BF16, tag=f"g{b}")
                    nc.gpsimd.indirect_dma_start(
                        out=g[:],
                        out_offset=None,
                        in_=w_p[b][:],
                        in_offset=bass.IndirectOffsetOnAxis(
                            ap=idx_sb[:, t : t + 1], axis=0
                        ),
                    )
                    lhsT = lp.tile([P, kc * P], BF16, tag=f"l{b}")
                    for k in range(kc):
                        cw = min(P, d - k * P)
                        trp = ps_tr.tile([P, P], BF16, tag="tr")
                        nc.tensor.transpose(
                            out=trp[:cw, :P],
                            in_=g[:, k * P : k * P + cw],
                            identity=ident[:],
                        )
                        if ncast % 2 == 0:
                            nc.vector.tensor_copy(
                                out=lhsT[:cw, k * P : (k + 1) * P], in_=trp[:cw, :P]
                            )
                        else:
                            nc.scalar.activation(
                                out=lhsT[:cw, k * P : (k + 1) * P],
                                in_=trp[:cw, :P],
                                func=mybir.ActivationFunctionType.Copy,
                            )
                        ncast += 1
                    mm0 = ps_mm.tile([P, 512], F32, tag="mm0")
                    mm1 = ps_mm.tile([P, 512], F32, tag="mm1")
                    mms = [mm0, mm1]
                    for k in range(kc):
                        cw = min(P, d - k * P)
                        for h in range(2):
                            nc.tensor.matmul(
                                mms[h][:, :],
                                lhsT[:cw, k * P : (k + 1) * P],
                                pt_sb[0:cw, pt_off + k * 1024 + h * 512 : pt_off + k * 1024 + (h + 1) * 512],
                                start=(k == 0),
                                stop=(k == kc - 1),
                            )
                    ob = t * D_PROJ
                    nc.vector.tensor_copy(out=obuf[:, ob : ob + 512], in_=mm0[:, :])
                    nc.scalar.activation(
                        out=obuf[:, ob + 512 : ob + 1024],
                        in_=mm1[:, :],
                        func=mybir.ActivationFunctionType.Copy,
                    )
                nc.sync.dma_start(
                    out=out_p[:, t0 : t0 + nt, :],
                    in_=obuf[:, t0 * D_PROJ : (t0 + nt) * D_PROJ],
                )

    nc.compile()
    return nc


def kernel(inp, emb0, emb1, emb2, emb3, proj0, proj1, proj2, proj3):
    embs = [np.asarray(e, dtype=np.float32) for e in (emb0, emb1, emb2, emb3)]
    projs = [proj0, proj1, proj2, proj3]
    v_emb = [e.shape[0] for e in embs]
    embs_bf = [e.astype(BF16NP) for e in embs]

    inp = np.asarray(inp)
    orig_shape = inp.shape
    flat = inp.reshape(-1).astype(np.int64)

    bucket = np.digitize(flat, CUTS[1:-1])  # 0..3
    local = flat - np.asarray(CUTS, dtype=np.int64)[bucket]

    # per bucket: sort by row, deal balanced contiguous chunks to cores
    core_chunks = {}
    for b in range(N_BUCKETS):
        pos = np.nonzero(bucket == b)[0]
        loc = np.clip(local[pos], 0, v_emb[b] - 1)
        srt = np.argsort(loc, kind="stable")
        pos, loc = pos[srt], loc[srt]
        n = len(pos)
        base, rem = divmod(n, N_CORES)
        ofs = 0
        chunks = []
        for c in range(N_CORES):
            cnt = base + (1 if c < rem else 0)
            chunks.append((loc[ofs : ofs + cnt], pos[ofs : ofs + cnt]))
            ofs += cnt
        core_chunks[b] = chunks

    # uniform SPMD shapes: per bucket, N idx slots (multiple of 128, padded
    # with idx 0) and W window rows (max span over cores)
    plan = {"N": {}, "W": {}, "tile_off": {}}
    to = 0
    for b in BUCKET_ORDER:
        maxn = max(len(core_chunks[b][c][0]) for c in range(N_CORES))
        plan["N"][b] = max(P, _cdiv(maxn, P) * P)
        maxw = 1
        for c in range(N_CORES):
            lc, _ = core_chunks[b][c]
            if len(lc):
                maxw = max(maxw, int(lc[-1]) - int(lc[0]) + 1)
        plan["W"][b] = maxw
        plan["tile_off"][b] = to
        to += plan["N"][b] // P
    plan["tiles_total"] = to

    # packed projection images: ptA = [b2 | b3 | b1 chunks], ptB = b0 chunks
    pt_scaled = [
        (np.asarray(projs[b], dtype=np.float32).T * EMB_SCALE) for b in range(N_BUCKETS)
    ]  # [d_b, 1024]
    plan["pt_off"] = {2: 0, 3: 1024, 1: 2048, 0: 0}
    ptA = np.zeros((P, 4096), dtype=np.float32)
    ptA[0:64, 0:1024] = pt_scaled[2]
    ptA[0:16, 1024:2048] = pt_scaled[3]
    ptA[:, 2048:3072] = pt_scaled[1][0:128]
    ptA[:, 3072:4096] = pt_scaled[1][128:256]
    ptB = np.zeros((P, 8 * 1024), dtype=np.float32)
    for k in range(8):
        ptB[:, k * 1024 : (k + 1) * 1024] = pt_scaled[0][k * P : (k + 1) * P]
    ptA = ptA.astype(BF16NP)
    ptB = ptB.astype(BF16NP)

    nc = _build_graph(plan)

    in_maps = []
    for c in range(N_CORES):
        im = {"ptA": ptA, "ptB": ptB}
        idx_img = np.zeros((P, plan["tiles_total"]), dtype=np.int32)
        for b in BUCKET_ORDER:
            lc, _ = core_chunks[b][c]
            start = int(lc[0]) if len(lc) else 0
            N = plan["N"][b]
            rel = np.zeros(N, dtype=np.int32)
            rel[: len(lc)] = (lc - start).astype(np.int32)
            t0 = plan["tile_off"][b]
            idx_img[:, t0 : t0 + N // P] = rel.reshape(N // P, P).T
            W = plan["W"][b]
            win = np.zeros((W, D_EMB[b]), dtype=BF16NP)
            take = min(W, v_emb[b] - start)
            win[:take] = embs_bf[b][start : start + take]
            im[f"w{b}"] = win
        im["idx"] = idx_img
        in_maps.append(im)

    res = run_bass_kernel_spmd(nc, in_maps, core_ids=list(range(N_CORES)))

    out_full = np.zeros((flat.shape[0], D_PROJ), dtype=np.float32)
    for c in range(N_CORES):
        shard = np.asarray(res.results[c]["out"])  # [128, T, 1024] bf16
        for b in BUCKET_ORDER:
            _, pc = core_chunks[b][c]
            if len(pc):
                t0 = plan["tile_off"][b]
                nt = plan["N"][b] // P
                blk = (
                    shard[:, t0 : t0 + nt, :]
                    .transpose(1, 0, 2)
                    .reshape(nt * P, D_PROJ)[: len(pc)]
                )
                out_full[pc] = blk.astype(np.float32)
    return out_full.reshape(*orig_shape, D_PROJ)


# revision 7
# speedup vs baseline: 1.5042x; 1.1861x over previous
"""Adaptive embedding lookup (4 vocab buckets, per-bucket projection) on 8 TRN2 cores.

Strategy: token-parallel SPMD, bf16 end-to-end, per-tile indirect gathers.

Host side: tokens are bucketed by vocab range, sorted by table row, and dealt
to the 8 cores as balanced *contiguous* chunks of the sorted order. Each core
gets a bf16 copy of exactly its span of each table (a "window") uploaded as an
input; gather indices are window-relative int32. Projections are
pre-transposed, EMB_SCALE-folded, and packed into two bf16 images.

Device side (per core):
  - per 128-token tile, one SWDGE indirect DMA gathers the tile's bf16 rows
    (~1.1us fixed engine cost each -- the pipeline bottleneck, overlapped
    with everything else)
  - PE transposes each gathered [128, d] tile (bf16: 1 cycle/row) and
    bf16 matmuls against the packed projections; PE has slack vs the gathers
  - PSUM -> SBUF bf16 casts split across Vector/Scalar into one persistent
    output image [128, T, 1024], written back with one DMA per bucket
A burst of dummy matmuls at graph start ramps the PE p-state clock
(0.65 -> 1.2 -> 2.4 GHz after 3us busy) while the first gathers land.
Host inverse-permutes the 8 bf16 shards into the full f32 output.
"""
import sys

import numpy as np

if "/opt/trn_rl_repo" not in sys.path:
    sys.path.insert(0, "/opt/trn_rl_repo")

import ml_dtypes  # noqa: E402
from concourse import bacc, bass, mybir, tile  # noqa: E402
from concourse.bass_utils import run_bass_kernel_spmd  # noqa: E402
from concourse.masks import make_identity  # noqa: E402

N_CORES = 8
P = 128
CUTS = [0, 20000, 40000, 200000, 267735]
N_BUCKETS = 4
D_PROJ = 1024
EMB_SCALE = float(D_PROJ) ** 0.5
D_EMB = [1024, 256, 64, 16]

F32 = mybir.dt.float32
BF16 = mybir.dt.bfloat16
I32 = mybir.dt.int32
BF16NP = ml_dtypes.bfloat16

# compute/gather order: b2 first (most tiles, smallest proj dependency),
# b0 last (needs the 2MB ptB image, which streams in behind ptA)
BUCKET_ORDER = [2, 3, 1, 0]


def _cdiv(a, b):
    return -(-a // b)


def _build_graph(plan):
    nc = bacc.Bacc(None, target_bir_lowering=False, debug=False)

    T = plan["tiles_total"]
    idx_p = nc.declare_dram_parameter("idx", [P, T], I32, isOutput=False)
    w_p = {}
    for b in range(N_BUCKETS):
        w_p[b] = nc.declare_dram_parameter(
            f"w{b}", [plan["W"][b], D_EMB[b]], BF16, isOutput=False
        )
    ptA_p = nc.declare_dram_parameter("ptA", [P, 4096], BF16, isOutput=False)
    ptB_p = nc.declare_dram_parameter("ptB", [P, 8 * 1024], BF16, isOutput=False)
    out_p = nc.declare_dram_parameter("out", [P, T, D_PROJ], BF16, isOutput=True)

    with tile.TileContext(nc) as tc:
        with (
            tc.tile_pool(name="persist", bufs=1) as pp,
            tc.tile_pool(name="gather", bufs=12) as gp,
            tc.tile_pool(name="lhsT", bufs=12) as lp,
            tc.tile_pool(name="ps_tr", bufs=2, space="PSUM") as ps_tr,
            tc.tile_pool(name="ps_mm", bufs=2, space="PSUM") as ps_mm,
            tc.tile_pool(name="ps_warm", bufs=1, space="PSUM") as ps_warm,
        ):
            # idx load first on the sync HWDGE queue (fast fixed overhead)
            idx_sb = pp.tile([P, T], I32)
            nc.sync.dma_start(out=idx_sb[:], in_=idx_p[:])

            ident = pp.tile([P, P], BF16)
            make_identity(nc, ident[:])

            # PE warmup: ramp the p-state clock while the first gathers land
            warm = pp.tile([P, 512], BF16, tag="warm")
            nc.vector.memset(warm[:], 0)
            wps = ps_warm.tile([P, 512], F32, tag="warm_ps")
            for _ in range(10):
                nc.tensor.matmul(wps[:], warm[:, :P], warm[:], start=True, stop=True)

            ptA_sb = pp.tile([P, 4096], BF16, tag="ptA")
            nc.scalar.dma_start(out=ptA_sb[:], in_=ptA_p[:])
            ptB_sb = pp.tile([P, 8 * 1024], BF16, tag="ptB")
            nc.scalar.dma_start(out=ptB_sb[:], in_=ptB_p[:])

            # persistent output image, one big writeback per bucket
            obuf = pp.tile([P, T * D_PROJ], BF16, tag="obuf")

            order = []
            nts = {b: plan["N"][b] // P for b in BUCKET_ORDER}
            order += [(2, j) for j in range(min(2, nts[2]))]
            order += [(0, j) for j in range(nts[0])]
            order += [(1, j) for j in range(nts[1])]
            order += [(2, j) for j in range(2, nts[2])]
            order += [(3, j) for j in range(nts[3])]

            ncast = 0
            for b, j in order:
                d = D_EMB[b]
                kc = _cdiv(d, P)
                nt = nts[b]
                t0 = plan["tile_off"][b]
                pt_sb = ptB_sb if b == 0 else ptA_sb
                pt_off = plan["pt_off"][b]
                if True:
                    t = t0 + j
                    g = gp.tile([P, d], BF16, tag=f"g{b}")
                    nc.gpsimd.indirect_dma_start(
                        out=g[:],
                        out_offset=None,
                        in_=w_p[b][:],
                        in_offset=bass.IndirectOffsetOnAxis(
                            ap=idx_sb[:, t : t + 1], axis=0
                        ),
                    )
                    lhsT = lp.tile([P, kc * P], BF16, tag=f"l{b}")
                    for k in range(kc):
                        cw = min(P, d - k * P)
                        trp = ps_tr.tile([P, P], BF16, tag="tr")
                        nc.tensor.transpose(
                            out=trp[:cw, :P],
                            in_=g[:, k * P : k * P + cw],
                            identity=ident[:],
                        )
                        if ncast % 2 == 0:
                            nc.vector.tensor_copy(
                                out=lhsT[:cw, k * P : (k + 1) * P], in_=trp[:cw, :P]
                            )
                        else:
                            nc.scalar.activation(
                                out=lhsT[:cw, k * P : (k + 1) * P],
                                in_=trp[:cw, :P],
                                func=mybir.ActivationFunctionType.Copy,
                            )
                        ncast += 1
                    mm0 = ps_mm.tile([P, 512], F32, tag="mm0")
                    mm1 = ps_mm.tile([P, 512], F32, tag="mm1")
                    mms = [mm0, mm1]
                    for k in range(kc):
                        cw = min(P, d - k * P)
                        for h in range(2):
                            nc.tensor.matmul(
                                mms[h][:, :],
                                lhsT[:cw, k * P : (k + 1) * P],
                                pt_sb[0:cw, pt_off + k * 1024 + h * 512 : pt_off + k * 1024 + (h + 1) * 512],
                                start=(k == 0),
                                stop=(k == kc - 1),
                            )
                    ob = t * D_PROJ
                    nc.vector.tensor_copy(out=obuf[:, ob : ob + 512], in_=mm0[:, :])
                    nc.scalar.activation(
                        out=obuf[:, ob + 512 : ob + 1024],
                        in_=mm1[:, :],
                        func=mybir.ActivationFunctionType.Copy,
                    )
            for b in BUCKET_ORDER:
                nt = nts[b]
                t0 = plan["tile_off"][b]
                nc.sync.dma_start(
                    out=out_p[:, t0 : t0 + nt, :],
                    in_=obuf[:, t0 * D_PROJ : (t0 + nt) * D_PROJ],
                )

    nc.compile()
    return nc


def kernel(inp, emb0, emb1, emb2, emb3, proj0, proj1, proj2, proj3):
    embs = [np.asarray(e, dtype=np.float32) for e in (emb0, emb1, emb2, emb3)]
    projs = [proj0, proj1, proj2, proj3]
    v_emb = [e.shape[0] for e in embs]
    embs_bf = [e.astype(BF16NP) for e in embs]

    inp = np.asarray(inp)
    orig_shape = inp.shape
    flat = inp.reshape(-1).astype(np.int64)

    bucket = np.digitize(flat, CUTS[1:-1])  # 0..3
    local = flat - np.asarray(CUTS, dtype=np.int64)[bucket]

    # per bucket: sort by row, deal balanced contiguous chunks to cores
    core_chunks = {}
    for b in range(N_BUCKETS):
        pos = np.nonzero(bucket == b)[0]
        loc = np.clip(local[pos], 0, v_emb[b] - 1)
        srt = np.argsort(loc, kind="stable")
        pos, loc = pos[srt], loc[srt]
        n = len(pos)
        base, rem = divmod(n, N_CORES)
        ofs = 0
        chunks = []
        for c in range(N_CORES):
            cnt = base + (1 if c < rem else 0)
            chunks.append((loc[ofs : ofs + cnt], pos[ofs : ofs + cnt]))
            ofs += cnt
        core_chunks[b] = chunks

    # uniform SPMD shapes: per bucket, N idx slots (multiple of 128, padded
    # with idx 0) and W window rows (max span over cores)
    plan = {"N": {}, "W": {}, "tile_off": {}}
    to = 0
    for b in BUCKET_ORDER:
        maxn = max(len(core_chunks[b][c][0]) for c in range(N_CORES))
        plan["N"][b] = max(P, _cdiv(maxn, P) * P)
        maxw = 1
        for c in range(N_CORES):
            lc, _ = core_chunks[b][c]
            if len(lc):
                maxw = max(maxw, int(lc[-1]) - int(lc[0]) + 1)
        plan["W"][b] = maxw
        plan["tile_off"][b] = to
        to += plan["N"][b] // P
    plan["tiles_total"] = to

    # packed projection images: ptA = [b2 | b3 | b1 chunks], ptB = b0 chunks
    pt_scaled = [
        (np.asarray(projs[b], dtype=np.float32).T * EMB_SCALE) for b in range(N_BUCKETS)
    ]  # [d_b, 1024]
    plan["pt_off"] = {2: 0, 3: 1024, 1: 2048, 0: 0}
    ptA = np.zeros((P, 4096), dtype=np.float32)
    ptA[0:64, 0:1024] = pt_scaled[2]
    ptA[0:16, 1024:2048] = pt_scaled[3]
    ptA[:, 2048:3072] = pt_scaled[1][0:128]
    ptA[:, 3072:4096] = pt_scaled[1][128:256]
    ptB = np.zeros((P, 8 * 1024), dtype=np.float32)
    for k in range(8):
        ptB[:, k * 1024 : (k + 1) * 1024] = pt_scaled[0][k * P : (k + 1) * P]
    ptA = ptA.astype(BF16NP)
    ptB = ptB.astype(BF16NP)

    nc = _build_graph(plan)

    in_maps = []
    for c in range(N_CORES):
        im = {"ptA": ptA, "ptB": ptB}
        idx_img = np.zeros((P, plan["tiles_total"]), dtype=np.int32)
        for b in BUCKET_ORDER:
            lc, _ = core_chunks[b][c]
            start = int(lc[0]) if len(lc) else 0
            N = plan["N"][b]
            rel = np.zeros(N, dtype=np.int32)
            rel[: len(lc)] = (lc - start).astype(np.int32)
            t0 = plan["tile_off"][b]
            idx_img[:, t0 : t0 + N // P] = rel.reshape(N // P, P).T
            W = plan["W"][b]
            win = np.zeros((W, D_EMB[b]), dtype=BF16NP)
            take = min(W, v_emb[b] - start)
            win[:take] = embs_bf[b][start : start + take]
            im[f"w{b}"] = win
        im["idx"] = idx_img
        in_maps.append(im)

    res = run_bass_kernel_spmd(nc, in_maps, core_ids=list(range(N_CORES)))

    out_full = np.zeros((flat.shape[0], D_PROJ), dtype=np.float32)
    for c in range(N_CORES):
        shard = np.asarray(res.results[c]["out"])  # [128, T, 1024] bf16
        for b in BUCKET_ORDER:
            _, pc = core_chunks[b][c]
            if len(pc):
                t0 = plan["tile_off"][b]
                nt = plan["N"][b] // P
                blk = (
                    shard[:, t0 : t0 + nt, :]
                    .transpose(1, 0, 2)
                    .reshape(nt * P, D_PROJ)[: len(pc)]
                )
                out_full[pc] = blk.astype(np.float32)
    return out_full.reshape(*orig_shape, D_PROJ)
